# revision 8
# baseline (speedup 1.0000x reference)
"""GAT (2-layer, 4-head, N=4096) Bass kernel for 8 Trainium2 NeuronCores.

Sharding: destination-node rows are split across the 8 cores (512 rows each).
x / weights are replicated; each core receives its own column-block of adj^T.

Per-core layout ("layout B"): attention logits are built TRANSPOSED, as
e^T[j, i] tiles of [128 source nodes (partitions), P local dest rows (free)].
 - e^T = (f1_bcast + f2_scalar) + adjneg  in ONE fused DVE scalar_tensor_tensor
   (adjneg = 0 for edges, -300 for non-edges; exp flushes masked entries to
   ~1e-26 exactly like the reference's -9e15 masking flushes them to 0).
 - leaky-relu on ACT (Prelu table, same ACT table-set as Exp) or on DVE
   (fused (x*0.2) max x), selectable per-chunk for engine load balancing.
 - p = exp(lr - 10) -> bf16 (the -10 shift cancels in softmax, keeps row sums
   within DVE reciprocal range).
 - att @ h needs the contraction index j on partitions -> p^T is already in
   the right orientation: hp^T[f, i] = sum_j h[j, f] p^T[j, i] accumulated in
   PSUM over 32 j-chunks.  An extra ones-column in h yields the softmax row
   sums for free; normalization is a rank-1 broadcast matmul + one multiply.
Layer 2 needs h2 = hcat @ W_out for ALL nodes on every core: each core
computes its local rows and one AllGather of [512, 65] bf16 (h2 | f2) shares
them.  Everything else is row-local.
"""
import sys

sys.path.insert(0, "/opt/trn_rl_repo")

import numpy as np

import concourse.bass as bass
import concourse.mybir as mybir
import concourse.tile as tile
from concourse.alu_op_type import AluOpType

F32 = mybir.dt.float32
BF16 = mybir.dt.bfloat16
U8 = mybir.dt.uint8
AF = mybir.ActivationFunctionType
ALPHA = 0.2
EXP_SHIFT = -10.0  # softmax-invariant shift, keeps row sums < 2^42


def _split_drain_waits(nc, max_waits=1):
    """walrus CoreV3 CTRL lowering accepts only one sem wait per Drain;
    split the tile-generated end-of-kernel drain into a chain of drains."""
    n = 0
    for fn in nc.m.functions:
        for blk in fn.blocks:
            i = 0
            while i < len(blk.instructions):
                inst = blk.instructions[i]
                si = inst.sync_info
                if (isinstance(inst, mybir.InstDrain) and si is not None
                        and len(si.on_wait) > max_waits):
                    waits = list(si.on_wait)
                    si.on_wait = waits[:max_waits]
                    rest = waits[max_waits:]
                    chunks = [rest[j:j + max_waits]
                              for j in range(0, len(rest), max_waits)]
                    for ci, ch in enumerate(chunks):
                        pre = mybir.InstDrain(
                            name=f"{inst.name}-ws{n}-{ci}",
                            engine=inst.engine, ins=[], outs=[],
                            sync_info=mybir.SyncInfo(on_wait=ch, on_update=[]))
                        nc.register_instruction(pre)
                        blk.instructions.insert(i, pre)
                        i += 1
                    n += 1
                i += 1
    return n


def build_gat(N=4096, F=64, H=4, FP=64, NCLS=64, NCORES=8, use_prelu=True,
              prelu_frac=1.0):
    """Build the SPMD Bass graph (identical on every core)."""
    P = N // NCORES     # local destination rows (free dim of e^T tiles)
    C = N // 128        # source-node chunks
    CL = P // 128       # local node chunks
    HF = H * FP         # 256
    KH = HF // 128      # 2 k-halves for layer-2 contraction
    assert P % 128 == 0 and HF % 128 == 0 and P <= 512

    nc = bass.Bass()
    xT_d = nc.declare_dram_parameter("xT", (F, N), F32, isOutput=False)
    xTl_d = nc.declare_dram_parameter("xTloc", (F, P), F32, isOutput=False)
    adj_d = nc.declare_dram_parameter("adjTu8", (N, P), U8, isOutput=False)
    Wall_d = nc.declare_dram_parameter("Wall", (F, HF), F32, isOutput=False)
    WTall_d = nc.declare_dram_parameter("WTall", (FP, H * F), F32, isOutput=False)
    aTh_d = nc.declare_dram_parameter("aTh", (FP, 2 * H), F32, isOutput=False)
    Wo_d = nc.declare_dram_parameter("Wo", (HF, NCLS), F32, isOutput=False)
    WoT_d = nc.declare_dram_parameter("WoT", (NCLS, HF), F32, isOutput=False)
    ao_d = nc.declare_dram_parameter("ao", (NCLS, 2), F32, isOutput=False)
    out_d = nc.declare_dram_parameter("outT", (NCLS, P), F32, isOutput=True)

    cc_in = nc.dram_tensor("cc_in", (P, NCLS + 1), BF16)
    cc_out = nc.dram_tensor("cc_out", (N, NCLS + 1), BF16, addr_space="Shared")

    with tile.TileContext(nc) as tc:
        with tc.tile_pool(name="const", bufs=1) as cp, \
             tc.tile_pool(name="stage", bufs=1) as sp, \
             tc.tile_pool(name="work", bufs=3) as wp, \
             tc.tile_pool(name="post", bufs=2) as pp, \
             tc.tile_pool(name="psacc", bufs=2, space="PSUM") as ps_acc, \
             tc.tile_pool(name="psf1b", bufs=1, space="PSUM") as ps_f1b, \
             tc.tile_pool(name="psmisc", bufs=2, space="PSUM") as ps_m:

            # ---------------- staging / constants ----------------
            xT_f = sp.tile([F, N], F32, tag="xTf")
            xTl_f = sp.tile([F, P], F32, tag="xTlf")
            adju8 = sp.tile([128, C, P], U8, tag="adju8")
            WTall_f = sp.tile([FP, H * F], F32, tag="WTallf")
            aTh_f = sp.tile([FP, 2 * H], F32, tag="aThf")
            Wo_f = sp.tile([128, KH, NCLS], F32, tag="Wof")
            WoT_f = sp.tile([NCLS, HF], F32, tag="WoTf")
            ao_f = sp.tile([NCLS, 2], F32, tag="aof")
            Wall_f = sp.tile([F, HF], F32, tag="Wallf")

            xT_b = cp.tile([F, N], BF16, tag="xTb")
            xTl_b = cp.tile([F, P], BF16, tag="xTlb")
            WallE = cp.tile([F, HF + 2 * H], BF16, tag="WallE")
            Waco_f = cp.tile([F, 2 * H], F32, tag="Wacof")
            adjn = cp.tile([128, C, P], BF16, tag="adjn")
            h_sb = cp.tile([128, H, C, FP + 1], BF16, tag="h_sb")
            f12c = cp.tile([128, C, 2 * H], F32, tag="f12c")
            f12r = cp.tile([2 * H, P], F32, tag="f12r")
            f1rows = cp.tile([1, H, P], F32, tag="f1rows")
            hcatT = cp.tile([128, KH, P], BF16, tag="hcatT")
            h2p = cp.tile([128, C, NCLS + 1], BF16, tag="h2p")
            f2c2 = cp.tile([128, C], F32, tag="f2c2")
            f1r2 = cp.tile([1, P], F32, tag="f1r2")
            WoE = cp.tile([128, KH, NCLS + 1], BF16, tag="WoE")
            w1a_b = cp.tile([128, KH], BF16, tag="w1a")
            ones = cp.tile([1, 128], F32, tag="ones")
            neg10 = cp.tile([128, 1], F32, tag="neg10")
            h2g = cp.tile([128, CL, NCLS + 1], BF16, tag="h2g")
            F1b = cp.tile([128, P], F32, tag="F1b")
            F1b2 = cp.tile([128, P], F32, tag="F1b2")

            # ---------------- input DMAs ----------------
            nc.sync.dma_start(out=xT_f[:], in_=xT_d[:])
            nc.sync.dma_start(out=xTl_f[:], in_=xTl_d[:])
            nc.sync.dma_start(out=Wall_f[:], in_=Wall_d[:])
            nc.sync.dma_start(out=WTall_f[:], in_=WTall_d[:])
            nc.sync.dma_start(out=aTh_f[:], in_=aTh_d[:])
            nc.sync.dma_start(out=WoT_f[:], in_=WoT_d[:])
            nc.sync.dma_start(out=ao_f[:], in_=ao_d[:])
            for k in range(KH):
                nc.sync.dma_start(out=Wo_f[:, k, :], in_=Wo_d[128 * k:128 * (k + 1), :])
            for jc in range(C):
                nc.sync.dma_start(out=adju8[:, jc, :],
                                  in_=adj_d[128 * jc:128 * (jc + 1), :])

            nc.vector.memset(ones[:], 1.0)
            nc.vector.memset(neg10[:], EXP_SHIFT)

            # ---------------- prep: casts & small matmuls ----------------
            nc.vector.tensor_copy(xT_b[:], xT_f[:])
            nc.vector.tensor_copy(xTl_b[:], xTl_f[:])
            nc.scalar.copy(out=WallE[:, 0:HF], in_=Wall_f[:])

            # Wa columns: W_h @ a1_h, W_h @ a2_h  (f32 exact)
            waco_ps = ps_m.tile([F, 2 * H], F32, tag="misc")
            for h in range(H):
                for k in range(2):
                    nc.tensor.matmul(
                        waco_ps[:, 2 * h + k:2 * h + k + 1],
                        WTall_f[:, F * h:F * (h + 1)],
                        aTh_f[:, 2 * h + k:2 * h + k + 1],
                        start=True, stop=True)
            nc.vector.tensor_copy(Waco_f[:], waco_ps[:])
            nc.scalar.copy(out=WallE[:, HF:HF + 2 * H], in_=waco_ps[:])

            # adjneg = adj*300 - 300  (0 on edges, -300 off)
            for jc in range(C):
                nc.vector.tensor_scalar(out=adjn[:, jc, :], in0=adju8[:, jc, :],
                                        scalar1=300.0, scalar2=-300.0,
                                        op0=AluOpType.mult, op1=AluOpType.add)

            # h (node-on-partition, all heads) + ones col ; f1/f2 per node
            for jc in range(C):
                hp_ps = ps_m.tile([128, HF], F32, tag="misc")
                nc.tensor.matmul(hp_ps[:],
                                 xT_b[:, 128 * jc:128 * (jc + 1)],
                                 WallE[:, 0:HF], start=True, stop=True)
                nc.scalar.copy(out=h_sb[:, :, jc, 0:FP],
                               in_=hp_ps[:].rearrange("p (h f) -> p h f", h=H))
                f12_ps = ps_m.tile([128, 2 * H], F32, tag="misc")
                nc.tensor.matmul(f12_ps[:],
                                 xT_f[:, 128 * jc:128 * (jc + 1)],
                                 Waco_f[:], start=True, stop=True)
                nc.vector.tensor_copy(f12c[:, jc, :], f12_ps[:])
            nc.vector.memset(h_sb[:, :, :, FP], 1.0)

            # f1/f2 rows over local columns (f32)
            f12r_ps = ps_m.tile([2 * H, P], F32, tag="misc")
            nc.tensor.matmul(f12r_ps[:], Waco_f[:], xTl_f[:], start=True, stop=True)
            nc.vector.tensor_copy(f12r[:], f12r_ps[:])
            for h in range(H):
                nc.sync.dma_start(out=f1rows[0:1, h, :],
                                  in_=f12r[2 * h:2 * h + 1, :])

            # ---------------- layer 1: 4 heads ----------------
            def attention_rows(F1b_t, f2_scalar_col, lhs_tile, hp_acc, tag):
                """One layer-unit: for all C chunks build p^T and accumulate
                hp^T = [h|1]^T p^T (rows 0..M-1 = feats, row M = softmax sum)."""
                for jc in range(C):
                    e_m = wp.tile([128, P], F32, tag="e_m")
                    p_b = wp.tile([128, P], BF16, tag="p_b")
                    nc.vector.scalar_tensor_tensor(
                        out=e_m[:], in0=F1b_t[:], scalar=f2_scalar_col(jc),
                        in1=adjn[:, jc, :], op0=AluOpType.add, op1=AluOpType.add)
                    if use_prelu and (jc % 4) < int(round(prelu_frac * 4)):
                        lr = wp.tile([128, P], F32, tag="lr")
                        nc.scalar.activation(out=lr[:], in_=e_m[:], func=AF.Prelu,
                                             alpha=ALPHA)
                    else:
                        lr = wp.tile([128, P], F32, tag="lr")
                        nc.vector.scalar_tensor_tensor(
                            out=lr[:], in0=e_m[:], scalar=ALPHA, in1=e_m[:],
                            op0=AluOpType.mult, op1=AluOpType.max)
                    nc.scalar.activation(out=p_b[:], in_=lr[:], func=AF.Exp,
                                         bias=neg10[:, 0:1])
                    nc.tensor.matmul(hp_acc[:], lhs_tile(jc), p_b[:],
                                     start=(jc == 0), stop=(jc == C - 1))

            def normalize(hp_acc, M, out_tile):
                """out = hp[0:M] / hp[M]  (rank-1 broadcast of reciprocal)."""
                rinv = pp.tile([1, P], F32, tag="rinv")
                nc.vector.reciprocal(rinv[:], hp_acc[M:M + 1, :])
                R_ps = ps_m.tile([128, P], F32, tag="Rps")
                nc.tensor.matmul(R_ps[0:M, :], ones[:, 0:M], rinv[:],
                                 start=True, stop=True)
                R_sb = pp.tile([M, P], F32, tag="Rsb")
                nc.vector.tensor_copy(R_sb[:], R_ps[0:M, :])
                nc.vector.tensor_tensor(out=out_tile[:], in0=hp_acc[0:M, :],
                                        in1=R_sb[:], op=AluOpType.mult)

            for h in range(H):
                # F1b = broadcast of f1 row (f32 rank-1 matmul)
                f1b_ps = ps_f1b.tile([128, P], F32, tag="f1bps")
                nc.tensor.matmul(f1b_ps[:], ones[:], f1rows[0:1, h, :],
                                 start=True, stop=True)
                nc.vector.tensor_copy(F1b[:], f1b_ps[:])

                hp_acc = ps_acc.tile([FP + 1, P], F32, tag="hp")
                attention_rows(F1b, lambda jc, h=h: f12c[:, jc, 2 * h + 1:2 * h + 2],
                               lambda jc, h=h: h_sb[:, h, jc, :], hp_acc, f"l1h{h}")

                u = pp.tile([FP, P], F32, tag="u")
                normalize(hp_acc, FP, u)
                # elu(u) = (relu(u) - 1) + exp(min(u, 0))
                t2 = pp.tile([FP, P], F32, tag="t2")
                nc.vector.tensor_scalar_min(out=t2[:], in0=u[:], scalar1=0.0)
                t3 = pp.tile([FP, P], F32, tag="t3")
                nc.scalar.activation(out=t3[:], in_=t2[:], func=AF.Exp)
                r1m = pp.tile([FP, P], F32, tag="r1m")
                nc.vector.tensor_scalar(out=r1m[:], in0=u[:], scalar1=0.0,
                                        scalar2=-1.0, op0=AluOpType.max,
                                        op1=AluOpType.add)
                nc.vector.tensor_tensor(
                    out=hcatT[FP * (h % 2):FP * (h % 2) + FP, h // 2, :],
                    in0=t3[:], in1=r1m[:], op=AluOpType.add)

            # ---------------- layer 2 prep ----------------
            # w1a/w2a = W_out @ a1/a2  (f32)
            for k in range(KH):
                w12_ps = ps_m.tile([128, 2], F32, tag="misc")
                for j in range(2):
                    nc.tensor.matmul(w12_ps[:, j:j + 1],
                                     WoT_f[:, 128 * k:128 * (k + 1)],
                                     ao_f[:, j:j + 1],
                                     start=True, stop=True)
                nc.scalar.copy(out=WoE[:, k, 0:NCLS], in_=Wo_f[:, k, :])
                nc.scalar.copy(out=WoE[:, k, NCLS:NCLS + 1], in_=w12_ps[:, 1:2])
                nc.vector.tensor_copy(w1a_b[:, k:k + 1], w12_ps[:, 0:1])

            # local h2 rows (+f2 col) and the f1 row
            for lc in range(CL):
                h2_ps = ps_m.tile([128, NCLS + 1], F32, tag="misc")
                for k in range(KH):
                    nc.tensor.matmul(h2_ps[:], hcatT[:, k, 128 * lc:128 * (lc + 1)],
                                     WoE[:, k, :], start=(k == 0), stop=(k == KH - 1))
                nc.vector.tensor_copy(h2g[:, lc, :], h2_ps[:])
            f1r2_ps = ps_m.tile([1, P], F32, tag="misc")
            for k in range(KH):
                nc.tensor.matmul(f1r2_ps[:], w1a_b[:, k:k + 1], hcatT[:, k, :],
                                 start=(k == 0), stop=(k == KH - 1))
            nc.vector.tensor_copy(f1r2[:], f1r2_ps[:])

            # ---------------- AllGather h2|f2 ----------------
            for lc in range(CL):
                nc.sync.dma_start(out=cc_in[128 * lc:128 * (lc + 1), :],
                                  in_=h2g[:, lc, :])
            nc.gpsimd.collective_compute(
                "AllGather", AluOpType.bypass,
                replica_groups=[list(range(NCORES))],
                ins=[cc_in[:]], outs=[cc_out[:]])
            for jc in range(C):
                nc.sync.dma_start(out=h2p[:, jc, :],
                                  in_=cc_out[128 * jc:128 * (jc + 1), :])
            nc.vector.tensor_copy(f2c2[:], h2p[:, :, NCLS])
            nc.vector.memset(h2p[:, :, NCLS], 1.0)

            # ---------------- layer 2 ----------------
            f1b2_ps = ps_f1b.tile([128, P], F32, tag="f1bps")
            nc.tensor.matmul(f1b2_ps[:], ones[:], f1r2[:], start=True, stop=True)
            nc.vector.tensor_copy(F1b2[:], f1b2_ps[:])

            hp2_acc = ps_acc.tile([NCLS + 1, P], F32, tag="hp")
            attention_rows(F1b2, lambda jc: f2c2[:, jc:jc + 1],
                           lambda jc: h2p[:, jc, :], hp2_acc, "l2")

            outT_sb = pp.tile([NCLS, P], F32, tag="outT")
            normalize(hp2_acc, NCLS, outT_sb)
            nc.sync.dma_start(out=out_d[:], in_=outT_sb[:])

    import bass_rust as _bass_rust
    _bass_rust.generate_event_semaphores(nc)
    nc.finalize()
    return nc


def make_in_maps(x, W_heads, a_heads, W_out, a_out, adj, ncores=8):
    """Pure layout transforms (transpose / slice / dtype) -> per-core inputs."""
    N, F = x.shape
    H = W_heads.shape[0]
    P = N // ncores
    xT = np.ascontiguousarray(x.T.astype(np.float32))
    adjT = adj.T.astype(np.uint8)
    Wall = np.ascontiguousarray(
        np.concatenate([W_heads[h] for h in range(H)], axis=1).astype(np.float32))
    WTall = np.ascontiguousarray(
        np.concatenate([W_heads[h].T for h in range(H)], axis=1).astype(np.float32))
    FPh = a_heads.shape[1] // 2
    aTh = np.ascontiguousarray(
        a_heads.reshape(H, 2, FPh).transpose(2, 0, 1).reshape(FPh, 2 * H)
        .astype(np.float32))
    Wo = np.ascontiguousarray(W_out.astype(np.float32))
    WoT = np.ascontiguousarray(W_out.T.astype(np.float32))
    ao = np.ascontiguousarray(a_out.astype(np.float32).reshape(2, -1).T)
    in_maps = []
    for c in range(ncores):
        in_maps.append({
            "xT": xT,
            "xTloc": np.ascontiguousarray(xT[:, c * P:(c + 1) * P]),
            "adjTu8": np.ascontiguousarray(adjT[:, c * P:(c + 1) * P]),
            "Wall": Wall, "WTall": WTall, "aTh": aTh,
            "Wo": Wo, "WoT": WoT, "ao": ao,
        })
    return in_maps


_CACHE = {}


def _run(x, W_heads, a_heads, W_out, a_out, adj, trace=False, **bkw):
    from concourse.bass_utils import run_bass_kernel_spmd

    N, F = x.shape
    H, _, FP = W_heads.shape
    NCLS = W_out.shape[1]
    NCORES = 8
    key = (N, F, H, FP, NCLS) + tuple(sorted(bkw.items()))
    if key not in _CACHE:
        _CACHE[key] = build_gat(N=N, F=F, H=H, FP=FP, NCLS=NCLS, NCORES=NCORES,
                                **bkw)
    nc = _CACHE[key]
    in_maps = make_in_maps(x, W_heads, a_heads, W_out, a_out, adj, NCORES)
    res = run_bass_kernel_spmd(nc, in_maps, core_ids=list(range(NCORES)),
                               trace=trace)
    out = np.concatenate([res.results[c]["outT"].T for c in range(NCORES)], axis=0)
    return out.astype(np.float32), res


def kernel(x, W_heads, a_heads, W_out, a_out, adj):
    out, _ = _run(np.asarray(x), np.asarray(W_heads), np.asarray(a_heads),
                  np.asarray(W_out), np.asarray(a_out), np.asarray(adj))
    return out
